# revision 32
# baseline (speedup 1.0000x reference)
"""CausalPrefixAttention TRN2 Bass kernel (v2).

Full-input contract: kernel(**inputs) takes the complete tensors and returns
the complete [2, 1024, 1024] output. Internally shards (batch, head-group)
across 8 NeuronCores: core c handles batch c//4 and heads 4*(c%4) .. +4.

v2 design notes (cost-model driven):
- bf16 datapath: weights/activations bf16 (fp32 PSUM accumulation).
- norm_w folded into the projection weights on host; LN reduces to
  z = (x - mu) * rstd  (bias path available when norm_b != 0).
- LN stats via DVE bn_stats/bn_aggr; LN apply on GpSimd (frees ACT for exp).
- RoPE: 2 DVE muls (swap via negative-stride view) + GpSimd subtract.
- sim matmuls write 2-bank PSUM tiles; one wide exp per (head, jc) with
  causal column trimming.
- PV in q-major orientation (out [q, dh]) -> per-partition denominators,
  tiny reciprocals, per-partition normalize.
- x token tiles processed first so causal attention overlaps context
  projection; PE warmup transposes keep the p-state ramp going.
"""

import sys

for _p in ("/opt/trn_rl_repo", "/root/.axon_site/_ro/trn_rl_repo"):
    if _p not in sys.path:
        sys.path.append(_p)

import numpy as np

import concourse.bass as bass
import concourse.mybir as mybir
import concourse.tile as tile
from concourse import bacc, bass_utils


def _install_ntff_hook():
    """Provide antenv.axon_hooks (NTFF profiling shim) if the image lacks it."""
    try:
        from antenv import axon_hooks  # noqa: F401
        return
    except ImportError:
        pass
    import contextlib
    import ctypes
    import os
    import types

    so_path = "/opt/axon/libaxon_pjrt.so"
    hook = None
    if os.path.exists(so_path):
        lib = ctypes.CDLL(so_path)
        if hasattr(lib, "axon_start_nrt_profile"):
            lib.axon_start_nrt_profile.argtypes = [
                ctypes.POINTER(ctypes.c_int64), ctypes.c_size_t]
            lib.axon_start_nrt_profile.restype = ctypes.c_int64
            lib.axon_stop_nrt_profile.argtypes = [ctypes.c_char_p]
            lib.axon_stop_nrt_profile.restype = ctypes.c_int64

            @contextlib.contextmanager
            def hook(output_dir, device_ids):
                import jax
                jax.devices()
                if device_ids:
                    ids = (ctypes.c_int64 * len(device_ids))(*device_ids)
                    rc = lib.axon_start_nrt_profile(ids, len(device_ids))
                else:
                    rc = lib.axon_start_nrt_profile(None, 0)
                if rc != 0:
                    raise RuntimeError(f"axon_start_nrt_profile rc={rc}")
                try:
                    yield
                finally:
                    n = lib.axon_stop_nrt_profile(str(output_dir).encode())
                    print(f"ntff profile: {n} file(s) -> {output_dir}")

    mod = types.ModuleType("antenv.axon_hooks")
    mod.get_axon_ntff_profile_hook = lambda: hook
    mod.set_axon_ntff_profile_hook = lambda h: None
    sys.modules["antenv.axon_hooks"] = mod


_install_ntff_hook()

F32 = mybir.dt.float32
BF16 = mybir.dt.bfloat16
U8 = mybir.dt.uint8
AF = mybir.ActivationFunctionType
ALU = mybir.AluOpType

DIM = 1024
HEADS = 16
DH = 64
B = 2
N = 1024          # query tokens
CTX = 1024        # context tokens
J = CTX + N       # kv length
HPC = 4           # heads per core
INNER_C = HPC * DH  # 256 per-core inner width
SCALE = DH ** -0.5
LN_EPS = 1e-5
NEG = -1e30

N_CORES = 8
NT = N // 128      # 8 query-token tiles
JT = J // 128      # 16 kv tiles
DT = DIM // 128    # 8 d-chunks

N_WARMUP = 26      # PE warmup transposes to ride out the p-state ramp


def _build_program(share_kv_w, use_bias):
    nc = bacc.Bacc(
        "TRN2",
        target_bir_lowering=False,
        debug=False,
        enable_asserts=False,
        num_devices=N_CORES,
    )
    xb = nc.dram_tensor("xb", [N, DIM], F32, kind="ExternalInput").ap()
    cb = nc.dram_tensor("cb", [CTX, DIM], F32, kind="ExternalInput").ap()
    # weights packed partition-major on host: [128, DT*INNER_C] bf16
    wq = nc.dram_tensor("wq", [128, DT * INNER_C], BF16, kind="ExternalInput").ap()
    wkx = nc.dram_tensor("wkx", [128, DT * INNER_C], BF16, kind="ExternalInput").ap()
    wvx = nc.dram_tensor("wvx", [128, DT * INNER_C], BF16, kind="ExternalInput").ap()
    if share_kv_w:
        wkc, wvc = wkx, wvx
    else:
        wkc = nc.dram_tensor("wkc", [128, DT * INNER_C], BF16, kind="ExternalInput").ap()
        wvc = nc.dram_tensor("wvc", [128, DT * INNER_C], BF16, kind="ExternalInput").ap()
    wo = nc.dram_tensor("wo", [128, 2 * DIM], BF16, kind="ExternalInput").ap()
    # rotary tables packed [128, JT*32] / [128, JT*64] (sin | -sin)
    cost = nc.dram_tensor("cost", [128, JT * 32], F32, kind="ExternalInput").ap()
    sint = nc.dram_tensor("sint", [128, JT * 64], F32, kind="ExternalInput").ap()
    cmask = nc.dram_tensor("cmask", [128, CTX // 128], U8, kind="ExternalInput").ap()
    if use_bias:
        bvec = nc.dram_tensor("bvec", [1, 5 * INNER_C], BF16, kind="ExternalInput").ap()
    else:
        bvec = None
    y = nc.dram_tensor("y", [N, DIM], F32, kind="ExternalOutput").ap()

    with tile.TileContext(nc) as tc:
        _kernel_body(tc, xb, cb, wq, wkx, wvx, wkc, wvc, wo, cost, sint,
                     cmask, bvec, y)
    nc.finalize()
    return nc


def _kernel_body(tc, xb, cb, wq, wkx, wvx, wkc, wvc, wo, cost, sint,
                 cmask, bvec, y, dbg=None):
    nc = tc.nc
    ctx_lp = nc.allow_low_precision(reason="bf16 matmul operands; fp32 PSUM accumulation")
    ctx_lp.__enter__()

    mm = nc.tensor.matmul

    with (
        tc.tile_pool(name="consts", bufs=1) as cpool,
        tc.tile_pool(name="acts", bufs=1) as apool,
    ):
        # ---- constants ----------------------------------------------------
        onesb = cpool.tile([128, 128], BF16, tag="onesb", name="onesb")
        nc.vector.memset(onesb[:], 1.0)
        identb = cpool.tile([128, 128], BF16, tag="identb", name="identb")
        nc.gpsimd.affine_select(
            identb[:], onesb[:], pattern=[[1, 128]], base=0,
            channel_multiplier=-1, compare_op=ALU.is_equal, fill=0.0,
        )
        onecol = onesb[:, 0:1]   # [128,1] bf16 ones (denominator matmuls)

        eps_t = cpool.tile([128, 1], F32, tag="lneps", name="lneps")
        nc.vector.memset(eps_t[:], LN_EPS)

        cospk = cpool.tile([128, JT * 32], F32, tag="cospk", name="cospk")
        nc.sync.dma_start(cospk[:], cost[:])
        sinpk = cpool.tile([128, JT * 64], F32, tag="sinpk", name="sinpk")
        nc.sync.dma_start(sinpk[:], sint[:])

        mu = cpool.tile([128, CTX // 128], U8, tag="mu8", name="mu8")
        nc.sync.dma_start(mu[:], cmask[:])
        cmaddpk = cpool.tile([128, CTX // 128], F32, tag="cmaddpk", name="cmaddpk")
        nc.vector.tensor_scalar(
            cmaddpk[:], mu[:], scalar1=-NEG, scalar2=NEG, op0=ALU.mult, op1=ALU.add
        )
        cmadd = [cmaddpk[:, jc:jc + 1] for jc in range(CTX // 128)]

        # ---- weights ------------------------------------------------------
        wq_t = cpool.tile([128, DT * INNER_C], BF16, tag="wq", name="wq")
        nc.sync.dma_start(wq_t[:], wq[:])
        wkx_t = cpool.tile([128, DT * INNER_C], BF16, tag="wkx", name="wkx")
        nc.sync.dma_start(wkx_t[:], wkx[:])
        wvx_t = cpool.tile([128, DT * INNER_C], BF16, tag="wvx", name="wvx")
        nc.sync.dma_start(wvx_t[:], wvx[:])
        if wkc is wkx:
            wkc_t, wvc_t = wkx_t, wvx_t
        else:
            wkc_t = cpool.tile([128, DT * INNER_C], BF16, tag="wkc", name="wkc")
            nc.sync.dma_start(wkc_t[:], wkc[:])
            wvc_t = cpool.tile([128, DT * INNER_C], BF16, tag="wvc", name="wvc")
            nc.sync.dma_start(wvc_t[:], wvc[:])
        wo_t = cpool.tile([128, 2 * DIM], BF16, tag="wo", name="wo")
        nc.sync.dma_start(wo_t[:], wo[:])
        if bvec is not None:
            bv_t = cpool.tile([1, 5 * INNER_C], BF16, tag="bv", name="bv")
            nc.sync.dma_start(bv_t[:], bvec[:])
            bq_v = bv_t[:, 0 * INNER_C:1 * INNER_C]
            bkx_v = bv_t[:, 1 * INNER_C:2 * INNER_C]
            bvx_v = bv_t[:, 2 * INNER_C:3 * INNER_C]
            bkc_v = bv_t[:, 3 * INNER_C:4 * INNER_C]
            bvc_v = bv_t[:, 4 * INNER_C:5 * INNER_C]

        # ---- persistent activation tiles ---------------------------------
        # qT: [128 = 2hc x (2 heads x 64 dh), hc*1024 + q]
        qT = apool.tile([128, 2 * N], BF16, tag="qT", name="qT")
        kT = apool.tile([128, 2 * J], BF16, tag="kT", name="kT")
        # vAll: [128 j-tok, jc*260 + h*65 + (0:64 v | 64 ones)]
        vAll = apool.tile([128, JT * HPC * (DH + 1)], BF16, tag="vAll", name="vAll")
        v4 = vAll[:].rearrange("p (j h c) -> p j h c", j=JT, h=HPC)
        nc.vector.memset(v4[:, :, :, DH:DH + 1], 1.0)
        # causal P storage: per head, 8 jc tiles of [128 j, 1024 q] bf16
        Pc = [apool.tile([128, 8 * N], BF16, tag=f"Pc{h}", name=f"Pc{h}")
              for h in range(HPC)]
        woin = apool.tile([128, 2 * N], BF16, tag="woin", name="woin")
        # reciprocal staging [128, 16] (head-pair: h%2 * 8 + m)
        rcpt = apool.tile([128, 16], F32, tag="rcpt", name="rcpt")
        # attn normalize staging: pair of heads side by side per m
        stg = [apool.tile([128, 128], BF16, tag=f"stg{m}", name=f"stg{m}")
               for m in range(NT)]

        qT2 = qT[:].rearrange("p (h q) -> p h q", h=2)
        kT2 = kT[:].rearrange("p (h q) -> p h q", h=2)

        # ------------------------------------------------------------------
        P = {}  # phase-scoped psum pools, rebound per with-block below
        with (
            tc.tile_pool(name="xin", bufs=4) as xin_pool,
            tc.tile_pool(name="lnstat", bufs=4) as st_pool,
            tc.tile_pool(name="zb", bufs=2) as z_pool,
            tc.tile_pool(name="zctx", bufs=8) as zctx_pool,
            tc.tile_pool(name="zT", bufs=3) as zT_pool,
            tc.tile_pool(name="rope", bufs=3) as rp_pool,
            tc.tile_pool(name="ropeo", bufs=5) as ro_pool,
        ):
            def ln_tile(src, t, pool, tag, pre=None):
                """Load token tile t of src, return z [128,1024] bf16."""
                if pre is not None:
                    xt = pre
                else:
                    xt = xin_pool.tile([128, DIM], F32, tag="xt", name="xt")
                    nc.sync.dma_start(xt[:], src[128 * t:128 * (t + 1), :])
                bn6 = st_pool.tile([128, 12], F32, tag="bn6", name="bn6")
                nc.vector.bn_stats(bn6[:, 0:6], xt[:, 0:512])
                nc.vector.bn_stats(bn6[:, 6:12], xt[:, 512:1024])
                st2 = st_pool.tile([128, 2], F32, tag="st2", name="st2")
                nc.vector.bn_aggr(st2[:], bn6[:])
                std = st_pool.tile([128, 1], F32, tag="std", name="std")
                nc.scalar.activation(std[:], st2[:, 1:2], AF.Sqrt, bias=eps_t[:])
                rstd = st_pool.tile([128, 1], F32, tag="rstd", name="rstd")
                nc.vector.reciprocal(rstd[:], std[:])
                nmr = st_pool.tile([128, 1], F32, tag="nmr", name="nmr")
                nc.vector.tensor_scalar(
                    nmr[:], st2[:, 0:1], scalar1=-1.0, scalar2=rstd[:],
                    op0=ALU.mult, op1=ALU.mult,
                )
                z = pool.tile([128, DIM], BF16, tag=tag, name=tag)
                nc.gpsimd.tensor_scalar(
                    z[:], xt[:], scalar1=rstd[:], scalar2=nmr[:],
                    op0=ALU.mult, op1=ALU.add,
                )
                return z

            def transpose_z(z, on_act):
                """z [128 tok, 1024] bf16 -> zT [128 dim-chunks, tok] bf16."""
                ztp = P["zt"].tile([128, DIM], BF16, tag="ztp", name="ztp")
                for dc in range(DT):
                    nc.tensor.transpose(
                        ztp[:, 128 * dc:128 * (dc + 1)],
                        z[:, 128 * dc:128 * (dc + 1)], identb[:],
                    )
                zT = zT_pool.tile([128, DIM], BF16, tag="zT", name="zT")
                if on_act:
                    nc.scalar.copy(zT[:], ztp[:])
                else:
                    nc.vector.tensor_copy(zT[:], ztp[:])
                return zT

            def project(zT, w_t, b_ap):
                ps = P["proj"].tile([128, INNER_C], F32, tag="proj", name="proj")
                for dc in range(DT):
                    mm(
                        ps[:], zT[:, 128 * dc:128 * (dc + 1)],
                        w_t[:, INNER_C * dc:INNER_C * (dc + 1)],
                        start=(dc == 0),
                        stop=(dc == DT - 1 and b_ap is None),
                    )
                if b_ap is not None:
                    mm(ps[:], onesb[0:1, 0:128], b_ap, start=False, stop=True)
                return ps

            def rope(ps, jc):
                """psum [128 tok, 256] -> bf16 SBUF tile, rotary applied."""
                p4 = ps[:].rearrange("p (h t f) -> p h t f", h=HPC, t=2)
                cosb = (cospk[:, 32 * jc:32 * (jc + 1)]
                        .unsqueeze(1).unsqueeze(1).broadcast_to([128, HPC, 2, 32]))
                sinb = (sinpk[:, 64 * jc:64 * (jc + 1)]
                        .rearrange("p (t f) -> p t f", t=2)
                        .unsqueeze(1).broadcast_to([128, HPC, 2, 32]))
                c1 = rp_pool.tile([128, INNER_C], F32, tag="ropec1", name="ropec1")
                c14 = c1[:].rearrange("p (h t f) -> p h t f", h=HPC, t=2)
                nc.vector.tensor_mul(c14, p4, cosb)
                tmp = rp_pool.tile([128, INNER_C], F32, tag="ropetm", name="ropetm")
                t4 = tmp[:].rearrange("p (h t f) -> p h t f", h=HPC, t=2)
                nc.vector.tensor_mul(t4, p4[:, :, ::-1, :], sinb)
                out = ro_pool.tile([128, INNER_C], BF16, tag="ropeo", name="ropeo")
                nc.gpsimd.tensor_sub(out[:], c1[:], tmp[:])
                return out

            def transpose_qk(src, dst2, col, on_act):
                """src [128 tok, 256] bf16 -> dst2[:, hc, col:col+128]."""
                ktp = P["kt"].tile([128, 256], BF16, tag="ktp", name="ktp")
                nc.tensor.transpose(ktp[:, 0:128], src[:, 0:128], identb[:])
                nc.tensor.transpose(ktp[:, 128:256], src[:, 128:256], identb[:])
                dst = dst2[:, :, col:col + 128]
                srcv = ktp[:].rearrange("p (h q) -> p h q", h=2)
                if on_act:
                    nc.scalar.copy(dst, srcv)
                else:
                    nc.vector.tensor_copy(dst, srcv)

            def proj_k(zT, jc, w_t, b_ap):
                ps = project(zT, w_t, b_ap)
                return rope(ps, jc)

            def proj_v(zT, jc, w_t, b_ap):
                ps = project(zT, w_t, b_ap)
                nc.scalar.copy(
                    v4[:, jc, :, 0:DH],
                    ps[:].rearrange("p (h f) -> p h f", h=HPC),
                )

            def proj_q(zT, t):
                ps = project(zT, wq_t, bq_v if bvec is not None else None)
                return rope(ps, t + CTX // 128)

            def sim_head(h, jc, lo):
                """sim psum [128 j, q in lo:1024] for head h, kv tile jc."""
                hb = 64 * (h % 2)
                hc = h // 2
                kslc = kT[hb:hb + 64, 2048 * hc + 128 * jc:2048 * hc + 128 * (jc + 1)]
                q_h = qT[hb:hb + 64, 1024 * hc:1024 * (hc + 1)]
                sp = P["sim"].tile([128, 1024], F32, tag="sim", name="sim")
                if lo < 512:
                    mm(sp[:, lo:512], kslc, q_h[:, lo:512], start=True, stop=True)
                    mm(sp[:, 512:1024], kslc, q_h[:, 512:1024], start=True, stop=True)
                else:
                    mm(sp[:, lo:1024], kslc, q_h[:, lo:1024], start=True, stop=True)
                return sp

            # ---- phase A-x: x tiles -> q, kx, vx (jc = 8+t) ---------------
            bkx = bkx_v if bvec is not None else None
            bvx = bvx_v if bvec is not None else None
            bkc = bkc_v if bvec is not None else None
            bvc = bvc_v if bvec is not None else None
            # prefetch the first input tiles ahead of the weight megaloads
            pre = []
            for src, t in ((xb, 0), (xb, 1), (cb, 0)):
                xt = xin_pool.tile([128, DIM], F32, tag="xt", name="xt")
                nc.sync.dma_start(xt[:], src[128 * t:128 * (t + 1), :])
                pre.append(xt)
            z_ctx = [None] * (CTX // 128)
            with (
                tc.tile_pool(name="proj_psA", bufs=3, space="PSUM") as proj_a,
                tc.tile_pool(name="zt_psA", bufs=2, space="PSUM") as zt_a,
                tc.tile_pool(name="kt_psA", bufs=2, space="PSUM") as kt_a,
            ):
                P["proj"], P["zt"], P["kt"] = proj_a, zt_a, kt_a
                pend = None   # (qs, ks, t) awaiting PE transposes
                for t in range(NT):
                    z = ln_tile(xb, t, z_pool, "z", pre=pre[t] if t < 2 else None)
                    zT = transpose_z(z, on_act=True)
                    if pend is not None:
                        transpose_qk(pend[0], qT2, 128 * pend[2], on_act=True)
                        transpose_qk(pend[1], kT2, 128 * (pend[2] + CTX // 128),
                                     on_act=True)
                    qs = proj_q(zT, t)
                    ks = proj_k(zT, t + CTX // 128, wkx_t, bkx)
                    proj_v(zT, t + CTX // 128, wvx_t, bvx)
                    pend = (qs, ks, t)
                    # hoisted ctx layernorm: all ACT Sqrt before the first Exp
                    z_ctx[t] = ln_tile(cb, t, zctx_pool, "zc",
                                       pre=pre[2] if t == 0 else None)
                transpose_qk(pend[0], qT2, 128 * pend[2], on_act=True)
                transpose_qk(pend[1], kT2, 128 * (pend[2] + CTX // 128),
                             on_act=True)

            # ---- interleaved: ctx projections + causal attention ----------
            def causal_unit(h, jc):
                lo = 128 * (jc - 8)
                sp = sim_head(h, jc, lo)
                pdst = Pc[h][:, 1024 * (jc - 8):1024 * (jc - 7)]
                nc.scalar.activation(pdst[:, lo:1024], sp[:, lo:1024], AF.Exp)
                nc.gpsimd.affine_select(
                    pdst[:, lo:lo + 128], pdst[:, lo:lo + 128],
                    pattern=[[1, 128]], base=0, channel_multiplier=-1,
                    compare_op=ALU.is_ge, fill=0.0,
                )

            cunits = [(h, jc) for h in range(HPC) for jc in range(8, JT)]
            with (
                tc.tile_pool(name="proj_psB", bufs=2, space="PSUM") as proj_b,
                tc.tile_pool(name="zt_psB", bufs=1, space="PSUM") as zt_b,
                tc.tile_pool(name="kt_psB", bufs=1, space="PSUM") as kt_b,
                tc.tile_pool(name="sim_psum", bufs=2, space="PSUM") as sim_b,
            ):
                P["proj"], P["zt"], P["kt"], P["sim"] = proj_b, zt_b, kt_b, sim_b
                pend = None   # (ks, t) awaiting PE transpose
                for t in range(CTX // 128):
                    zT = transpose_z(z_ctx[t], on_act=False)
                    if pend is not None:
                        transpose_qk(pend[0], kT2, 128 * pend[1], on_act=False)
                    for h, jc in cunits[4 * t:4 * t + 2]:
                        causal_unit(h, jc)
                    ks = proj_k(zT, t, wkc_t, bkc)
                    proj_v(zT, t, wvc_t, bvc)
                    for h, jc in cunits[4 * t + 2:4 * t + 4]:
                        causal_unit(h, jc)
                    pend = (ks, t)
                transpose_qk(pend[0], kT2, 128 * pend[1], on_act=False)

        # ---- phase B-ctx: context attention + pv --------------------------
        with (
            tc.tile_pool(name="sim_psum2", bufs=2, space="PSUM") as sim_psum2,
            tc.tile_pool(name="pv_psum", bufs=1, space="PSUM") as pv_psum,
            tc.tile_pool(name="dn_psum", bufs=1, space="PSUM") as dn_psum,
            tc.tile_pool(name="wt_psum", bufs=2, space="PSUM") as wt_psum,
            tc.tile_pool(name="pctx", bufs=4) as pctx_pool,
        ):
            def sim_head2(h, jc):
                hb = 64 * (h % 2)
                hc = h // 2
                kslc = kT[hb:hb + 64, 2048 * hc + 128 * jc:2048 * hc + 128 * (jc + 1)]
                q_h = qT[hb:hb + 64, 1024 * hc:1024 * (hc + 1)]
                sp = sim_psum2.tile([128, 1024], F32, tag="sim2", name="sim2")
                mm(sp[:, 0:512], kslc, q_h[:, 0:512], start=True, stop=True)
                mm(sp[:, 512:1024], kslc, q_h[:, 512:1024], start=True, stop=True)
                return sp

            def emit_pv(pvp, dnp, h, jc, pt):
                vslc = vAll[:, 260 * jc + 65 * h:260 * jc + 65 * h + 64]
                for m in range(NT):
                    # start=True zeroes the whole 2KB psum zero-region, so
                    # only the very first matmul into each bank starts.
                    first = (jc == 0 and m == 0)
                    pslc = pt[:, 128 * m:128 * (m + 1)]
                    mm(pvp[:, 64 * m:64 * (m + 1)], pslc, vslc,
                       start=first, stop=False)
                    mm(dnp[:, m:m + 1], pslc, onecol,
                       start=first, stop=False)

            for h in range(HPC):
                hp = h // 2
                pvp = pv_psum.tile([128, 512], F32, tag="pv", name="pv")
                dnp = dn_psum.tile([128, 8], F32, tag="dn", name="dn")
                hist = []
                for jc in range(CTX // 128):
                    # software pipeline depth 2: sim(jc) issues on PE before
                    # pv(jc-2), whose exp finished long ago -> no PE stalls
                    sp = sim_head2(h, jc)
                    pt = pctx_pool.tile([128, 1024], BF16, tag="pt", name="pt")
                    nc.scalar.activation(pt[:], sp[:], AF.Exp, bias=cmadd[jc])
                    hist.append((jc, pt))
                    if len(hist) >= 3:
                        pj, ppt = hist.pop(0)
                        emit_pv(pvp, dnp, h, pj, ppt)
                for pj, ppt in hist:
                    emit_pv(pvp, dnp, h, pj, ppt)
                # causal pv from saved P
                for m in range(NT):
                    for jc in range(8, 9 + m):
                        pslc = Pc[h][:, 1024 * (jc - 8) + 128 * m:
                                     1024 * (jc - 8) + 128 * (m + 1)]
                        vslc = vAll[:, 260 * jc + 65 * h:260 * jc + 65 * h + 64]
                        last = (m == NT - 1 and jc == 8 + m)
                        mm(pvp[:, 64 * m:64 * (m + 1)], pslc, vslc,
                           start=False, stop=last)
                        mm(dnp[:, m:m + 1], pslc, onecol,
                           start=False, stop=last)
                if dbg is not None and h == 0 and "d_pv0" in dbg:
                    with tc.tile_pool(name="dbgpv", bufs=1) as dp:
                        t1 = dp.tile([128, 512], F32, tag="dpv", name="dpv")
                        nc.vector.tensor_copy(t1[:], pvp[:])
                        nc.sync.dma_start(dbg["d_pv0"][:], t1[:])
                        t2 = dp.tile([128, 8], F32, tag="ddn", name="ddn")
                        nc.vector.tensor_copy(t2[:], dnp[:])
                        nc.sync.dma_start(dbg["d_dn0"][:], t2[:])
                # normalize into pair staging tiles
                rc = rcpt[:, 8 * (h % 2):8 * (h % 2) + 8]
                nc.vector.reciprocal(rc, dnp[:])
                for m in range(NT):
                    nc.vector.tensor_scalar_mul(
                        stg[m][:, 64 * (h % 2):64 * (h % 2) + 64],
                        pvp[:, 64 * m:64 * (m + 1)],
                        rc[:, m:m + 1],
                    )
                if h % 2 == 1:
                    wtps = [wt_psum.tile([128, 512], BF16, tag="wtp", name="wtp")
                            for _ in range(2)]
                    for m in range(NT):
                        nc.tensor.transpose(
                            wtps[m // 4][:, 128 * (m % 4):128 * (m % 4 + 1)],
                            stg[m][:], identb[:],
                        )
                    nc.vector.tensor_copy(woin[:, 1024 * hp:1024 * hp + 512], wtps[0][:])
                    nc.vector.tensor_copy(woin[:, 1024 * hp + 512:1024 * (hp + 1)], wtps[1][:])

        # ---- phase C: output projection -----------------------------------
        with (
            tc.tile_pool(name="wo_psum", bufs=2, space="PSUM") as wo_psum,
            tc.tile_pool(name="outsb", bufs=2) as out_pool,
        ):
            for m in range(NT):
                ps = wo_psum.tile([128, DIM], F32, tag="wops", name="wops")
                for f in range(2):
                    for hp in range(2):
                        mm(
                            ps[:, 512 * f:512 * (f + 1)],
                            woin[:, 1024 * hp + 128 * m:1024 * hp + 128 * (m + 1)],
                            wo_t[:, 1024 * hp + 512 * f:1024 * hp + 512 * (f + 1)],
                            start=(hp == 0), stop=(hp == 1),
                        )
                ot = out_pool.tile([128, DIM], F32, tag="osb", name="osb")
                if m % 2 == 0:
                    nc.scalar.copy(ot[:], ps[:])
                else:
                    nc.vector.tensor_copy(ot[:], ps[:])
                nc.gpsimd.dma_start(y[128 * m:128 * (m + 1), :], ot[:])

        if dbg is not None:
            with tc.tile_pool(name="dbg", bufs=1) as dbg_pool:
                for name, src in [("d_qT", qT), ("d_kT", kT), ("d_v", vAll),
                                  ("d_P0", Pc[0]), ("d_P1", Pc[1]),
                                  ("d_woin", woin)]:
                    if name not in dbg:
                        continue
                    w = src.shape[1]
                    cvt = dbg_pool.tile([128, w], F32, tag=f"cv{name}",
                                        name=f"cv{name}")
                    nc.vector.tensor_copy(cvt[:], src[:])
                    nc.sync.dma_start(dbg[name][:], cvt[:])
    ctx_lp.__exit__(None, None, None)


_PROGRAMS = {}
_LAST_RESULTS = None


def _get_program(share_kv_w, use_bias):
    key = (share_kv_w, use_bias)
    if key not in _PROGRAMS:
        _PROGRAMS[key] = _build_program(share_kv_w, use_bias)
    return _PROGRAMS[key]


def _pack_rows(a):
    # [DT*128, W] -> [128, DT*W] partition-major
    k, w = a.shape[0] // 128, a.shape[1]
    return np.ascontiguousarray(
        a.reshape(k, 128, w).transpose(1, 0, 2).reshape(128, k * w))


def kernel(x, context, context_mask, rotary_pos_emb, norm_w, norm_b,
           cnorm_w, cnorm_b, Wq, Wkv, Wo, bo, _trace=False):
    global _LAST_RESULTS
    x = np.ascontiguousarray(np.asarray(x, dtype=np.float32))
    context = np.ascontiguousarray(np.asarray(context, dtype=np.float32))
    rot = np.asarray(rotary_pos_emb, dtype=np.float32)
    norm_w = np.asarray(norm_w, dtype=np.float32)
    norm_b = np.asarray(norm_b, dtype=np.float32)
    cnorm_w = np.asarray(cnorm_w, dtype=np.float32)
    cnorm_b = np.asarray(cnorm_b, dtype=np.float32)
    Wq = np.asarray(Wq, dtype=np.float32)
    Wkv = np.asarray(Wkv, dtype=np.float32)
    Wo = np.asarray(Wo, dtype=np.float32)

    import ml_dtypes
    bf = ml_dtypes.bfloat16

    share_kv_w = bool(np.array_equal(norm_w, cnorm_w))
    use_bias = not (np.all(norm_b == 0.0) and np.all(cnorm_b == 0.0))

    s = np.sin(rot[:, :32])
    cost = _pack_rows(np.cos(rot[:, :32]))
    sint = _pack_rows(np.concatenate([s, -s], axis=1))
    mask_u8 = np.asarray(context_mask).reshape(B, CTX // 128, 128).view(np.uint8)
    mask_u8 = [np.ascontiguousarray(mask_u8[b].T) for b in range(B)]

    in_maps = []
    for c in range(N_CORES):
        b, hg = divmod(c, HEADS // HPC)
        lo = DH * HPC * hg
        wq_s = (norm_w[:, None] * Wq[:, lo:lo + INNER_C]) * SCALE
        wkx_s = norm_w[:, None] * Wkv[:, lo:lo + INNER_C]
        wvx_s = norm_w[:, None] * Wkv[:, HEADS * DH + lo:HEADS * DH + lo + INNER_C]
        m = {
            "xb": x[b],
            "cb": context[b],
            "wq": _pack_rows(wq_s).astype(bf),
            "wkx": _pack_rows(wkx_s).astype(bf),
            "wvx": _pack_rows(wvx_s).astype(bf),
            "wo": _pack_rows(Wo[lo:lo + INNER_C, :]).astype(bf),
            "cost": cost, "sint": sint,
            "cmask": mask_u8[b],
        }
        if not share_kv_w:
            wkc_s = cnorm_w[:, None] * Wkv[:, lo:lo + INNER_C]
            wvc_s = cnorm_w[:, None] * Wkv[:, HEADS * DH + lo:HEADS * DH + lo + INNER_C]
            m["wkc"] = _pack_rows(wkc_s).astype(bf)
            m["wvc"] = _pack_rows(wvc_s).astype(bf)
        if use_bias:
            bq = (norm_b @ Wq[:, lo:lo + INNER_C]) * SCALE
            bkx = norm_b @ Wkv[:, lo:lo + INNER_C]
            bvx = norm_b @ Wkv[:, HEADS * DH + lo:HEADS * DH + lo + INNER_C]
            bkc = cnorm_b @ Wkv[:, lo:lo + INNER_C]
            bvc = cnorm_b @ Wkv[:, HEADS * DH + lo:HEADS * DH + lo + INNER_C]
            m["bvec"] = np.concatenate([bq, bkx, bvx, bkc, bvc])[None, :].astype(bf)
        in_maps.append(m)

    nc = _get_program(share_kv_w, use_bias)
    res = bass_utils.run_bass_kernel_spmd(
        nc, in_maps, core_ids=list(range(N_CORES)), trace=_trace,
    )
    _LAST_RESULTS = res
    out = np.zeros((B, N, DIM), dtype=np.float32)
    for c in range(N_CORES):
        out[c // (HEADS // HPC)] += res.results[c]["y"]
    out += np.asarray(bo, dtype=np.float32)
    return out
